# revision 1
# baseline (speedup 1.0000x reference)
"""Bidirectional sigmoid-LSTM on 8 trn2 cores.

Sharding: hidden dim 1024 split 8 ways (128 hidden units per core per
direction).  Each core holds the 512 gate columns (4 gates x 128) of W/U
for its slice, computes its h-slice each step, and broadcasts it to all
8 cores via remote SBUF DMA.  Forward and backward direction interleave
on every engine so one direction's epilogue/exchange hides under the
other's matmuls.

Everything on-chip is transposed: gates live as (128, 2) per-gate tiles
(partition = gate column, free = batch), produced directly by matmuls
with the U-tile stationary, so the epilogue runs on (128, 8) tiles and
h needs no transpose before broadcast.
"""

import sys

sys.path.insert(0, "/opt/trn_rl_repo")

import numpy as np
import ml_dtypes

import concourse.bass as bass
import concourse.bacc as bacc
import concourse.mybir as mybir

D = 1024
NC = 8          # cores
KC = 8          # contraction chunks of 128
G = 4           # gates (i, f, g, o)
MS = 128        # my hidden-slice width

BF16 = mybir.dt.bfloat16
F32 = mybir.dt.float32
SIG = mybir.ActivationFunctionType.Sigmoid


def build_kernel(T: int) -> bass.Bass:
    nc = bacc.Bacc()

    xt_d = nc.declare_dram_parameter("xt", [128, KC * 2 * T], BF16, isOutput=False)
    wu_d = nc.declare_dram_parameter("wu", [128, 4 * 4096], BF16, isOutput=False)
    bias_d = nc.declare_dram_parameter("zb", [128, 8], F32, isOutput=False)
    out_f = nc.declare_dram_parameter("out_f", [128, 2 * T], BF16, isOutput=True)
    out_b = nc.declare_dram_parameter("out_b", [128, 2 * T], BF16, isOutput=True)

    # collective bounce buffers (double-buffered per direction)
    ccin = {(d, p): nc.dram_tensor("ccin_%s%d" % (d, p), [128, 2], BF16)
            for d in "fb" for p in (0, 1)}
    ccout = {(d, p): nc.dram_tensor("ccout_%s%d" % (d, p), [NC * 128, 2], BF16,
                                    addr_space="Shared")
             for d in "fb" for p in (0, 1)}

    ctxs = []

    def alloc(cm):
        v = cm.__enter__()
        ctxs.append(cm)
        return v

    # ---- SBUF ----
    xt_sb = alloc(nc.sbuf_tensor([128, KC * 2 * T], BF16))
    wu_sb = alloc(nc.sbuf_tensor([128, 4 * 4096], BF16))
    bias_sb = alloc(nc.sbuf_tensor([128, 8], F32))
    z0t = {d: alloc(nc.sbuf_tensor([128, 8 * T], F32)) for d in "fb"}
    hist = {d: alloc(nc.sbuf_tensor([128, 2 * T], BF16)) for d in "fb"}
    hbuf = {(d, p): alloc(nc.sbuf_tensor([128, 2 * NC], BF16))
            for d in "fb" for p in (0, 1)}
    z_sb = {d: alloc(nc.sbuf_tensor([128, 8], F32)) for d in "fb"}
    s_sb = {d: alloc(nc.sbuf_tensor([128, 8], F32)) for d in "fb"}
    c_sb = {d: alloc(nc.sbuf_tensor([128, 2], F32)) for d in "fb"}
    sc_sb = {d: alloc(nc.sbuf_tensor([128, 2], F32)) for d in "fb"}
    ig_sb = {d: alloc(nc.sbuf_tensor([128, 2], F32)) for d in "fb"}
    fc_sb = {d: alloc(nc.sbuf_tensor([128, 2], F32)) for d in "fb"}

    # ---- PSUM ----
    psum_pre = [alloc(nc.psum_tensor([128, 512], F32)) for _ in range(2)]
    psum = {(d, p): alloc(nc.psum_tensor([128, 8], F32))
            for d in "fb" for p in (0, 1)}

    # ---- semaphores ----
    sem = {}
    for name in ["load", "init", "pre", "pre_copy",
                 "pe_f", "pe_b", "zadd_f", "zadd_b", "sig_f", "sig_b",
                 "c_f", "c_b", "h_f", "h_b",
                 "harr_f0", "harr_f1", "harr_b0", "harr_b1",
                 "gdma_f", "gdma_b", "cc_f", "cc_b",
                 "outd", "ed_f", "ed_b"]:
        sem[name] = alloc(nc.semaphore(name))

    # weight block offsets inside wu: W_f, W_b, U_f, U_b
    WOFF = {"f": 0 * 4096, "b": 1 * 4096}
    UOFF = {"f": 2 * 4096, "b": 3 * 4096}

    # precompute tile schedule
    if (2 * T) % 512 == 0:
        TB, TBW = (2 * T) // 512, 512
    else:
        TB, TBW = 1, 2 * T
    pre_tiles = [(d, g, tb) for d in "fb" for g in range(G) for tb in range(TB)]

    with nc.Block() as block:

        @block.sync
        def _(sync):
            sync.dma_start(out=xt_sb[:], in_=xt_d[:]).then_inc(sem["load"], 16)
            sync.dma_start(out=wu_sb[:], in_=wu_d[:]).then_inc(sem["load"], 16)
            sync.dma_start(out=bias_sb[:], in_=bias_d[:]).then_inc(sem["load"], 16)
            sync.wait_ge(sem["h_f"], T)
            sync.dma_start(out=out_f[:], in_=hist["f"][:]).then_inc(sem["outd"], 16)
            sync.wait_ge(sem["h_b"], T)
            sync.dma_start(out=out_b[:], in_=hist["b"][:]).then_inc(sem["outd"], 16)
            sync.wait_ge(sem["outd"], 32)

        @block.tensor
        def _(pe):
            pe.wait_ge(sem["load"], 48)
            pe.wait_ge(sem["init"], 4)
            # ---- precompute Z0^T = W^T X^T ----
            for idx, (d, g, tb) in enumerate(pre_tiles):
                if idx >= 2:
                    pe.wait_ge(sem["pre_copy"], idx - 1)
                ps = psum_pre[idx % 2]
                for c in range(KC):
                    mm = pe.matmul(
                        out=ps[:, 0:TBW],
                        lhsT=wu_sb[:, WOFF[d] + c * 512 + g * 128:
                                   WOFF[d] + c * 512 + g * 128 + 128],
                        rhs=xt_sb[:, c * 2 * T + tb * TBW:
                                  c * 2 * T + (tb + 1) * TBW],
                        start=(c == 0), stop=(c == KC - 1),
                    )
                    if c == KC - 1:
                        mm.then_inc(sem["pre"], 1)
            # ---- recurrent steps ----
            for t in range(T):
                for d in "fb":
                    if t >= 1:
                        # step t consumes round t-1 (parity (t-1)%2)
                        pe.wait_ge(sem["harr_%s%d" % (d, (t - 1) % 2)],
                                   16 * ((t - 1) // 2 + 1))
                    if t >= 2:
                        pe.wait_ge(sem["zadd_" + d], t - 1)
                    ps = psum[(d, t % 2)]
                    hb = hbuf[(d, (t - 1) % 2)]
                    for g in range(G):
                        for c in range(KC):
                            mm = pe.matmul(
                                out=ps[:, 2 * g:2 * g + 2],
                                lhsT=wu_sb[:, UOFF[d] + c * 512 + g * 128:
                                           UOFF[d] + c * 512 + g * 128 + 128],
                                rhs=hb[:, 2 * c:2 * c + 2],
                                start=(c == 0), stop=(c == KC - 1),
                            )
                            if c == KC - 1 and g == G - 1:
                                mm.then_inc(sem["pe_" + d], 1)

        @block.vector
        def _(dve):
            for d in "fb":
                dve.memset(hbuf[(d, 1)][:], 0.0).then_inc(sem["init"], 1)
                dve.memset(c_sb[d][:], 0.0).then_inc(sem["init"], 1)
            # ---- precompute epilogue: psum -> z0t (strided) + bias ----
            for idx, (d, g, tb) in enumerate(pre_tiles):
                dve.wait_ge(sem["pre"], idx + 1)
                nt = TBW // 2
                src = psum_pre[idx % 2][:, 0:TBW].rearrange(
                    "p (t x) -> p t x", x=2)
                dst = z0t[d][:, :].rearrange("p (t x) -> p t x", x=8)[
                    :, tb * nt:(tb + 1) * nt, 2 * g:2 * g + 2]
                bcol = 4 * (0 if d == "f" else 1) + g
                dve.tensor_scalar_add(
                    out=dst, in0=src, scalar1=bias_sb[:, bcol:bcol + 1],
                ).then_inc(sem["pre_copy"], 1)
            # ---- recurrent epilogue ----
            for t in range(T):
                for d in "fb":
                    tt = t if d == "f" else T - 1 - t   # backward scans reversed
                    dve.wait_ge(sem["pe_" + d], t + 1)
                    if t >= 1:
                        dve.wait_ge(sem["sig_" + d], 2 * t - 1)  # WAR z_sb
                    dve.tensor_add(
                        out=z_sb[d][:], in0=psum[(d, t % 2)][:],
                        in1=z0t[d][:, 8 * tt:8 * tt + 8],
                    ).then_inc(sem["zadd_" + d], 1)
                    dve.wait_ge(sem["sig_" + d], 2 * t + 1)
                    dve.tensor_mul(out=ig_sb[d][:], in0=s_sb[d][:, 0:2],
                                   in1=s_sb[d][:, 4:6]).then_inc(sem["ed_" + d], 1)
                    if t >= 1:
                        dve.wait_ge(sem["c_" + d], t)            # RAW c_sb
                    dve.tensor_mul(out=fc_sb[d][:], in0=s_sb[d][:, 2:4],
                                   in1=c_sb[d][:]).then_inc(sem["ed_" + d], 1)
                    dve.wait_ge(sem["ed_" + d], 2 * t + 2)       # RAW ig/fc
                    dve.tensor_add(out=c_sb[d][:], in0=fc_sb[d][:],
                                   in1=ig_sb[d][:]).then_inc(sem["c_" + d], 1)
                    dve.wait_ge(sem["sig_" + d], 2 * t + 2)
                    dve.tensor_mul(
                        out=hist[d][:, 2 * tt:2 * tt + 2],
                        in0=s_sb[d][:, 6:8], in1=sc_sb[d][:],
                    ).then_inc(sem["h_" + d], 1)

        @block.scalar
        def _(act):
            for t in range(T):
                for d in "fb":
                    act.wait_ge(sem["zadd_" + d], t + 1)
                    if t >= 1:
                        act.wait_ge(sem["h_" + d], t)   # WAR s_sb
                    act.activation(out=s_sb[d][:], in_=z_sb[d][:], func=SIG
                                   ).then_inc(sem["sig_" + d], 1)
                    act.wait_ge(sem["c_" + d], t + 1)
                    act.activation(out=sc_sb[d][:], in_=c_sb[d][:], func=SIG
                                   ).then_inc(sem["sig_" + d], 1)

        @block.gpsimd
        def _(gp):
            rg = [list(range(NC))]
            for t in range(T - 1):   # last step's h is never consumed remotely
                p = t % 2
                for d in "fb":
                    tt = t if d == "f" else T - 1 - t
                    gp.wait_ge(sem["h_" + d], t + 1)
                    gp.dma_start(out=ccin[(d, p)][:],
                                 in_=hist[d][:, 2 * tt:2 * tt + 2]
                                 ).then_inc(sem["gdma_" + d], 16)
                    gp.wait_ge(sem["gdma_" + d], 16 * (t + 1))
                    if t >= 2:
                        # hbuf-refill DMA of t-2 must be done before ncfw
                        # overwrites ccout[p]
                        gp.wait_ge(sem["harr_%s%d" % (d, p)],
                                   16 * ((t - 2) // 2 + 1))
                    gp.collective_compute(
                        "AllGather",
                        mybir.AluOpType.bypass,
                        ins=[ccin[(d, p)][:]],
                        outs=[ccout[(d, p)][:]],
                        replica_groups=rg,
                    ).then_inc(sem["cc_" + d], 1)
                    gp.wait_ge(sem["cc_" + d], t + 1)
                    gp.dma_start(
                        out=hbuf[(d, p)][:, :].rearrange(
                            "p (c b) -> p c b", c=NC),
                        in_=ccout[(d, p)][:, :].rearrange(
                            "(c p) b -> p c b", p=128),
                    ).then_inc(sem["harr_%s%d" % (d, p)], 16)

    for cm in reversed(ctxs):
        cm.__exit__(None, None, None)
    nc.compile()
    return nc


# ---------------- host-side data prep / gather ----------------

def prepare_inputs(x, Wf, Uf, bf, Wb, Ub, bb, T):
    """Returns per-core input maps (list of dicts of np arrays)."""
    x = np.asarray(x, np.float32)
    X2 = x.reshape(2, T, D).transpose(1, 0, 2)          # (T, B, D)
    # xt[p, c*2T + 2t + b] = X2[t, b, 128c+p]
    xt = X2.transpose(2, 0, 1).reshape(D, 2 * T)        # (d, 2t+b)
    xt = xt.reshape(KC, 128, 2 * T).transpose(1, 0, 2).reshape(128, KC * 2 * T)
    xt = xt.astype(ml_dtypes.bfloat16)

    maps = []
    for k in range(NC):
        cols = np.concatenate(
            [np.arange(1024 * g + MS * k, 1024 * g + MS * k + MS)
             for g in range(G)])  # my 512 gate columns

        def pack(M):
            # -> (128, 4096): col c*512 + g*128 + j = M[128c+p, cols[g*128+j]]
            Mk = np.asarray(M, np.float32)[:, cols]     # (1024, 512)
            return (Mk.reshape(KC, 128, 512).transpose(1, 0, 2)
                    .reshape(128, KC * 512)).astype(ml_dtypes.bfloat16)

        wu = np.concatenate([pack(Wf), pack(Wb), pack(Uf), pack(Ub)], axis=1)
        zb = np.zeros((128, 8), np.float32)
        for gi, bv in ((0, bf), (1, bb)):
            bvk = np.asarray(bv, np.float32)
            for g in range(G):
                zb[:, 4 * gi + g] = bvk[1024 * g + MS * k: 1024 * g + MS * k + MS]
        maps.append({"xt": xt, "wu": wu, "zb": zb})
    return maps


def assemble_output(results, T):
    """results: list of per-core dicts with out_f/out_b (128, 2T) bf16."""
    hf = np.stack([np.asarray(r["out_f"], np.float32) for r in results])
    hb = np.stack([np.asarray(r["out_b"], np.float32) for r in results])

    def un(h):  # (8, 128, 2T) -> (B, T, 1024)
        h = h.reshape(NC, 128, T, 2)
        return h.transpose(3, 2, 0, 1).reshape(2, T, D)

    y = np.concatenate([un(hf), un(hb)], axis=-1)       # (2, T, 2048)
    return y.reshape(2, 1, T, 2 * D).astype(np.float32)


# ---------------- harness entry point ----------------

_CACHE = {}


def _get_nc(T):
    if T not in _CACHE:
        _CACHE[T] = build_kernel(T)
    return _CACHE[T]


def kernel(x, Wf, Uf, bf, Wb, Ub, bb):
    from concourse.bass_utils import run_bass_kernel_spmd

    T = x.shape[2]
    ncb = _get_nc(T)
    maps = prepare_inputs(x, Wf, Uf, bf, Wb, Ub, bb, T)
    res = run_bass_kernel_spmd(ncb, maps, list(range(NC)))
    return assemble_output(res.results, T)



# revision 2
# speedup vs baseline: 5.4007x; 5.4007x over previous
"""Bidirectional sigmoid-LSTM on 8 trn2 cores.

Sharding: hidden dim 1024 split 8 ways (128 hidden units per core per
direction).  Each core holds the 512 gate columns (4 gates x 128) of W/U
for its slice, computes its h-slice each step, and broadcasts it to all
8 cores via remote SBUF DMA.  Forward and backward direction interleave
on every engine so one direction's epilogue/exchange hides under the
other's matmuls.

Everything on-chip is transposed: gates live as (128, 2) per-gate tiles
(partition = gate column, free = batch), produced directly by matmuls
with the U-tile stationary, so the epilogue runs on (128, 8) tiles and
h needs no transpose before broadcast.
"""

import os
import sys

sys.path.insert(0, "/opt/trn_rl_repo")

# Persistent jax/XLA compilation cache: run_bass_kernel_spmd re-traces its
# jit closure every call, which otherwise re-runs the multi-second
# BIR-verify/walrus pipeline on each invocation.  With the persistent cache
# the compiled executable is fetched by HLO hash instead.
os.environ.setdefault("JAX_COMPILATION_CACHE_DIR", "/tmp/jaxcache")
os.environ.setdefault("JAX_PERSISTENT_CACHE_MIN_COMPILE_TIME_SECS", "1")
try:
    import jax

    jax.config.update("jax_compilation_cache_dir", "/tmp/jaxcache")
    jax.config.update("jax_persistent_cache_min_compile_time_secs", 1)
except Exception:
    pass

import numpy as np
import ml_dtypes

import concourse.bass as bass
import concourse.bacc as bacc
import concourse.mybir as mybir

D = 1024
NC = 8          # cores
KC = 8          # contraction chunks of 128
G = 4           # gates (i, f, g, o)
MS = 128        # my hidden-slice width

BF16 = mybir.dt.bfloat16
F32 = mybir.dt.float32
SIG = mybir.ActivationFunctionType.Sigmoid


def build_kernel(T: int) -> bass.Bass:
    nc = bacc.Bacc()

    xt_d = nc.declare_dram_parameter("xt", [128, KC * 2 * T], BF16, isOutput=False)
    wu_d = nc.declare_dram_parameter("wu", [128, 4 * 4096], BF16, isOutput=False)
    bias_d = nc.declare_dram_parameter("zb", [128, 8], F32, isOutput=False)
    out_f = nc.declare_dram_parameter("out_f", [128, 2 * T], BF16, isOutput=True)
    out_b = nc.declare_dram_parameter("out_b", [128, 2 * T], BF16, isOutput=True)

    # collective bounce buffers (double-buffered per direction)
    ccin = {(d, p): nc.dram_tensor("ccin_%s%d" % (d, p), [128, 2], BF16)
            for d in "fb" for p in (0, 1)}
    ccout = {(d, p): nc.dram_tensor("ccout_%s%d" % (d, p), [NC * 128, 2], BF16,
                                    addr_space="Shared")
             for d in "fb" for p in (0, 1)}

    ctxs = []

    def alloc(cm):
        v = cm.__enter__()
        ctxs.append(cm)
        return v

    # ---- SBUF ----
    xt_sb = alloc(nc.sbuf_tensor([128, KC * 2 * T], BF16))
    wu_sb = alloc(nc.sbuf_tensor([128, 4 * 4096], BF16))
    bias_sb = alloc(nc.sbuf_tensor([128, 8], F32))
    z0t = {d: alloc(nc.sbuf_tensor([128, 8 * T], F32)) for d in "fb"}
    hist = {d: alloc(nc.sbuf_tensor([128, 2 * T], BF16)) for d in "fb"}
    hbuf = {(d, p): alloc(nc.sbuf_tensor([128, 2 * NC], BF16))
            for d in "fb" for p in (0, 1)}
    z_sb = {d: alloc(nc.sbuf_tensor([128, 8], F32)) for d in "fb"}
    s_sb = {d: alloc(nc.sbuf_tensor([128, 8], F32)) for d in "fb"}
    c_sb = {d: alloc(nc.sbuf_tensor([128, 2], F32)) for d in "fb"}
    sc_sb = {d: alloc(nc.sbuf_tensor([128, 2], F32)) for d in "fb"}
    ig_sb = {d: alloc(nc.sbuf_tensor([128, 2], F32)) for d in "fb"}
    fc_sb = {d: alloc(nc.sbuf_tensor([128, 2], F32)) for d in "fb"}

    # ---- PSUM ----
    psum_pre = [alloc(nc.psum_tensor([128, 512], F32)) for _ in range(2)]
    psum = {(d, p): alloc(nc.psum_tensor([128, 8], F32))
            for d in "fb" for p in (0, 1)}

    # ---- semaphores ----
    sem = {}
    for name in ["load", "init", "pre", "pre_copy",
                 "pe_f", "pe_b", "zadd_f", "zadd_b", "sig_f", "sig_b",
                 "c_f", "c_b", "h_f", "h_b",
                 "harr_f0", "harr_f1", "harr_b0", "harr_b1",
                 "gdma_f", "gdma_b", "cc_f", "cc_b",
                 "outd", "ed_f", "ed_b"]:
        sem[name] = alloc(nc.semaphore(name))

    # weight block offsets inside wu: W_f, W_b, U_f, U_b
    WOFF = {"f": 0 * 4096, "b": 1 * 4096}
    UOFF = {"f": 2 * 4096, "b": 3 * 4096}

    # precompute tile schedule
    if (2 * T) % 512 == 0:
        TB, TBW = (2 * T) // 512, 512
    else:
        TB, TBW = 1, 2 * T
    pre_tiles = [(d, g, tb) for d in "fb" for g in range(G) for tb in range(TB)]

    with nc.Block() as block:

        @block.sync
        def _(sync):
            sync.dma_start(out=xt_sb[:], in_=xt_d[:]).then_inc(sem["load"], 16)
            sync.dma_start(out=wu_sb[:], in_=wu_d[:]).then_inc(sem["load"], 16)
            sync.dma_start(out=bias_sb[:], in_=bias_d[:]).then_inc(sem["load"], 16)
            sync.wait_ge(sem["h_f"], T)
            sync.dma_start(out=out_f[:], in_=hist["f"][:]).then_inc(sem["outd"], 16)
            sync.wait_ge(sem["h_b"], T)
            sync.dma_start(out=out_b[:], in_=hist["b"][:]).then_inc(sem["outd"], 16)
            sync.wait_ge(sem["outd"], 32)

        @block.tensor
        def _(pe):
            pe.wait_ge(sem["load"], 48)
            pe.wait_ge(sem["init"], 4)
            # ---- precompute Z0^T = W^T X^T ----
            for idx, (d, g, tb) in enumerate(pre_tiles):
                if idx >= 2:
                    pe.wait_ge(sem["pre_copy"], idx - 1)
                ps = psum_pre[idx % 2]
                for c in range(KC):
                    mm = pe.matmul(
                        out=ps[:, 0:TBW],
                        lhsT=wu_sb[:, WOFF[d] + c * 512 + g * 128:
                                   WOFF[d] + c * 512 + g * 128 + 128],
                        rhs=xt_sb[:, c * 2 * T + tb * TBW:
                                  c * 2 * T + (tb + 1) * TBW],
                        start=(c == 0), stop=(c == KC - 1),
                    )
                    if c == KC - 1:
                        mm.then_inc(sem["pre"], 1)
            # ---- recurrent steps ----
            for t in range(T):
                for d in "fb":
                    if t >= 1:
                        # step t consumes round t-1 (parity (t-1)%2)
                        pe.wait_ge(sem["harr_%s%d" % (d, (t - 1) % 2)],
                                   16 * ((t - 1) // 2 + 1))
                    if t >= 2:
                        pe.wait_ge(sem["zadd_" + d], t - 1)
                    ps = psum[(d, t % 2)]
                    hb = hbuf[(d, (t - 1) % 2)]
                    for g in range(G):
                        for c in range(KC):
                            mm = pe.matmul(
                                out=ps[:, 2 * g:2 * g + 2],
                                lhsT=wu_sb[:, UOFF[d] + c * 512 + g * 128:
                                           UOFF[d] + c * 512 + g * 128 + 128],
                                rhs=hb[:, 2 * c:2 * c + 2],
                                start=(c == 0), stop=(c == KC - 1),
                            )
                            if c == KC - 1 and g == G - 1:
                                mm.then_inc(sem["pe_" + d], 1)

        @block.vector
        def _(dve):
            for d in "fb":
                dve.memset(hbuf[(d, 1)][:], 0.0).then_inc(sem["init"], 1)
                dve.memset(c_sb[d][:], 0.0).then_inc(sem["init"], 1)
            # ---- precompute epilogue: psum -> z0t (strided) + bias ----
            for idx, (d, g, tb) in enumerate(pre_tiles):
                dve.wait_ge(sem["pre"], idx + 1)
                nt = TBW // 2
                src = psum_pre[idx % 2][:, 0:TBW].rearrange(
                    "p (t x) -> p t x", x=2)
                dst = z0t[d][:, :].rearrange("p (t x) -> p t x", x=8)[
                    :, tb * nt:(tb + 1) * nt, 2 * g:2 * g + 2]
                bcol = 4 * (0 if d == "f" else 1) + g
                dve.tensor_scalar_add(
                    out=dst, in0=src, scalar1=bias_sb[:, bcol:bcol + 1],
                ).then_inc(sem["pre_copy"], 1)
            # ---- recurrent epilogue ----
            for t in range(T):
                for d in "fb":
                    tt = t if d == "f" else T - 1 - t   # backward scans reversed
                    dve.wait_ge(sem["pe_" + d], t + 1)
                    if t >= 1:
                        dve.wait_ge(sem["sig_" + d], 2 * t - 1)  # WAR z_sb
                    dve.tensor_add(
                        out=z_sb[d][:], in0=psum[(d, t % 2)][:],
                        in1=z0t[d][:, 8 * tt:8 * tt + 8],
                    ).then_inc(sem["zadd_" + d], 1)
                    dve.wait_ge(sem["sig_" + d], 2 * t + 1)
                    dve.tensor_mul(out=ig_sb[d][:], in0=s_sb[d][:, 0:2],
                                   in1=s_sb[d][:, 4:6]).then_inc(sem["ed_" + d], 1)
                    if t >= 1:
                        dve.wait_ge(sem["c_" + d], t)            # RAW c_sb
                    dve.tensor_mul(out=fc_sb[d][:], in0=s_sb[d][:, 2:4],
                                   in1=c_sb[d][:]).then_inc(sem["ed_" + d], 1)
                    dve.wait_ge(sem["ed_" + d], 2 * t + 2)       # RAW ig/fc
                    dve.tensor_add(out=c_sb[d][:], in0=fc_sb[d][:],
                                   in1=ig_sb[d][:]).then_inc(sem["c_" + d], 1)
                    dve.wait_ge(sem["sig_" + d], 2 * t + 2)
                    dve.tensor_mul(
                        out=hist[d][:, 2 * tt:2 * tt + 2],
                        in0=s_sb[d][:, 6:8], in1=sc_sb[d][:],
                    ).then_inc(sem["h_" + d], 1)

        @block.scalar
        def _(act):
            for t in range(T):
                for d in "fb":
                    act.wait_ge(sem["zadd_" + d], t + 1)
                    if t >= 1:
                        act.wait_ge(sem["h_" + d], t)   # WAR s_sb
                    act.activation(out=s_sb[d][:], in_=z_sb[d][:], func=SIG
                                   ).then_inc(sem["sig_" + d], 1)
                    act.wait_ge(sem["c_" + d], t + 1)
                    act.activation(out=sc_sb[d][:], in_=c_sb[d][:], func=SIG
                                   ).then_inc(sem["sig_" + d], 1)

        @block.gpsimd
        def _(gp):
            rg = [list(range(NC))]
            for t in range(T - 1):   # last step's h is never consumed remotely
                p = t % 2
                for d in "fb":
                    tt = t if d == "f" else T - 1 - t
                    gp.wait_ge(sem["h_" + d], t + 1)
                    gp.dma_start(out=ccin[(d, p)][:],
                                 in_=hist[d][:, 2 * tt:2 * tt + 2]
                                 ).then_inc(sem["gdma_" + d], 16)
                    gp.wait_ge(sem["gdma_" + d], 16 * (t + 1))
                    if t >= 2:
                        # hbuf-refill DMA of t-2 must be done before ncfw
                        # overwrites ccout[p]
                        gp.wait_ge(sem["harr_%s%d" % (d, p)],
                                   16 * ((t - 2) // 2 + 1))
                    gp.collective_compute(
                        "AllGather",
                        mybir.AluOpType.bypass,
                        ins=[ccin[(d, p)][:]],
                        outs=[ccout[(d, p)][:]],
                        replica_groups=rg,
                    ).then_inc(sem["cc_" + d], 1)
                    gp.wait_ge(sem["cc_" + d], t + 1)
                    gp.dma_start(
                        out=hbuf[(d, p)][:, :].rearrange(
                            "p (c b) -> p c b", c=NC),
                        in_=ccout[(d, p)][:, :].rearrange(
                            "(c p) b -> p c b", p=128),
                    ).then_inc(sem["harr_%s%d" % (d, p)], 16)

    for cm in reversed(ctxs):
        cm.__exit__(None, None, None)
    nc.compile()
    return nc


# ---------------- host-side data prep / gather ----------------

def prepare_inputs(x, Wf, Uf, bf, Wb, Ub, bb, T):
    """Returns per-core input maps (list of dicts of np arrays)."""
    x = np.asarray(x, np.float32)
    X2 = x.reshape(2, T, D).transpose(1, 0, 2)          # (T, B, D)
    # xt[p, c*2T + 2t + b] = X2[t, b, 128c+p]
    xt = X2.transpose(2, 0, 1).reshape(D, 2 * T)        # (d, 2t+b)
    xt = xt.reshape(KC, 128, 2 * T).transpose(1, 0, 2).reshape(128, KC * 2 * T)
    xt = xt.astype(ml_dtypes.bfloat16)

    maps = []
    for k in range(NC):
        cols = np.concatenate(
            [np.arange(1024 * g + MS * k, 1024 * g + MS * k + MS)
             for g in range(G)])  # my 512 gate columns

        def pack(M):
            # -> (128, 4096): col c*512 + g*128 + j = M[128c+p, cols[g*128+j]]
            Mk = np.asarray(M, np.float32)[:, cols]     # (1024, 512)
            return (Mk.reshape(KC, 128, 512).transpose(1, 0, 2)
                    .reshape(128, KC * 512)).astype(ml_dtypes.bfloat16)

        wu = np.concatenate([pack(Wf), pack(Wb), pack(Uf), pack(Ub)], axis=1)
        zb = np.zeros((128, 8), np.float32)
        for gi, bv in ((0, bf), (1, bb)):
            bvk = np.asarray(bv, np.float32)
            for g in range(G):
                zb[:, 4 * gi + g] = bvk[1024 * g + MS * k: 1024 * g + MS * k + MS]
        maps.append({"xt": xt, "wu": wu, "zb": zb})
    return maps


def assemble_output(results, T):
    """results: list of per-core dicts with out_f/out_b (128, 2T) bf16."""
    hf = np.stack([np.asarray(r["out_f"], np.float32) for r in results])
    hb = np.stack([np.asarray(r["out_b"], np.float32) for r in results])

    def un(h):  # (8, 128, 2T) -> (B, T, 1024)
        h = h.reshape(NC, 128, T, 2)
        return h.transpose(3, 2, 0, 1).reshape(2, T, D)

    y = np.concatenate([un(hf), un(hb)], axis=-1)       # (2, T, 2048)
    return y.reshape(2, 1, T, 2 * D).astype(np.float32)


# ---------------- harness entry point ----------------

_CACHE = {}


def _get_nc(T):
    if T not in _CACHE:
        _CACHE[T] = build_kernel(T)
    return _CACHE[T]


def kernel(x, Wf, Uf, bf, Wb, Ub, bb):
    from concourse.bass_utils import run_bass_kernel_spmd

    T = x.shape[2]
    ncb = _get_nc(T)
    maps = prepare_inputs(x, Wf, Uf, bf, Wb, Ub, bb, T)
    res = run_bass_kernel_spmd(ncb, maps, list(range(NC)))
    return assemble_output(res.results, T)

